# revision 2
# baseline (speedup 1.0000x reference)
"""Trainium2 Bass kernel for nn_Decoder_29678224015654 — fp8 DoubleRow, v3.

v3 over v2:
- Conductor restructured: banked PSUM (one [128,512] bank per gate type,
  8 m-tiles of BL=64), banked activations, whole-layer cell math on
  DVE/Pool. L1 stays bf16 (zcon identity-injection carries cb1); L2 runs
  fp8 DoubleRow (cw2i/cw2h fp8 streams, cb2 via fp8 const chunk pair).
- Decoder gate PSUM tiles widened to [128,1024] (one per gate type,
  start_tensor_calc at both 2KB zero-regions), single activation per
  gate type and single tanh(c) per layer.
- zdec injected via fp8 ident-pair DoubleRow matmul (64 cyc vs 128).
- log_softmax batched: per step only the fc matmul + a Copy of logits to
  a bf16 history; the Exp/Ln chains run every 8 steps so the activation
  table (sigmoid/tanh) is not reloaded twice per step (1283 ns each).
- DMA queues balanced: SP carries decoder fp8 weights, gpsimd the
  conductor L1 streams, Activation the conductor L2 fp8 streams.
"""
import sys
sys.path.insert(0, '/opt/trn_rl_repo')

import numpy as np
import ml_dtypes

import concourse.bass as bass
import concourse.mybir as mybir
from concourse import bacc
from concourse.tile import TileContext
from concourse.bass_utils import run_bass_kernel_spmd

AF = mybir.ActivationFunctionType
BF = mybir.dt.bfloat16
F32 = mybir.dt.float32
F8 = mybir.dt.float8e4
PM = mybir.MatmulPerfMode.DoubleRow

SEQ, SUB, B, V = 32, 16, 512, 258
U = SEQ // SUB              # 2
Z, CH, CO, DH = 512, 1024, 512, 1024
NCORES = 8
BL = B // NCORES            # 64 batch rows per core
BD = BL * U                 # 128 decoder effective batch
VP = 384                    # V padded to 3 K-chunks
G = 4 * DH                  # 4096 gate cols
NT = G // 128               # 32 gate tiles
KH = DH // 128              # 8 K-chunks for hidden=1024
KZ = Z // 128               # 4
KV = VP // 128              # 3
KX = 6                      # W1x chunks: [W1x8(3), W1xr8(3)]
KI = 10                     # L2 h1 chunks: 8 + const pair (bias)
FCB = 8                     # softmax flush batch

bf16 = ml_dtypes.bfloat16
f8 = ml_dtypes.float8_e4m3


def build_nc(nsteps_cond=U, nsteps_dec=SUB, num_devices=NCORES):
    nc = bacc.Bacc("TRN2", target_bir_lowering=False, debug=False,
                   num_devices=num_devices)

    def inp(name, shape, dtype=BF):
        return nc.dram_tensor(name, list(shape), dtype, kind="ExternalInput")

    # fp8 decoder weights, m-tile-major, k-chunk pairs for DoubleRow
    d_w1h8 = inp("w1h8", [128, NT, KH, 128], F8)
    d_w2i8 = inp("w2i8", [128, NT, KI, 128], F8)   # chunk 8 = db2 row
    d_w2h8 = inp("w2h8", [128, NT, KH, 128], F8)
    d_w1i8 = inp("w1i8", [128, NT, KX, 128], F8)   # [W1x8, W1xr8]
    d_w1c8 = inp("w1c8", [128, NT, 6, 128], F8)    # chunk 4 = db1 row
    d_const8 = inp("const8", [128, 2, BD], F8)     # const pair (p0 = 1.0)
    # conductor: L1 bf16, L2 fp8
    d_cw1h = inp("cw1h", [NT, 128, CH])
    d_cw2i8 = inp("cw2i8", [NT, 128, KI, 128], F8)  # chunk 8 = cb2 row
    d_cw2h8 = inp("cw2h8", [NT, 128, KH, 128], F8)
    d_cw1i = inp("cw1i", [128, NT, 4, 128])        # bf16; chunk 3 = cb1 row
    d_ciw = inp("ciw", [128, KH, 5, 128])          # bf16; chunk 4 = cib row
    d_cow = inp("cow", [KZ, 128, KH + 1, 128])     # bf16; chunk 8 = cob row
    d_diw8 = inp("diw8", [128, KH, 6, 128], F8)    # chunk 4 = dib row
    d_fcw = inp("fcw", [KH, 128, V])

    d_id = inp("ident", [128, 128])                # bf16 identity
    d_id8p = inp("ident8p", [128, 2, 128], F8)     # [I, 0] fp8 pair
    d_fcb = inp("fcb", [1, V])                     # bf16
    d_ones = inp("ones", [1, 128])                 # bf16

    d_zt = inp("zt", [KZ + 1, 128, BL])            # z^T + one-hot chunk
    d_cin = inp("cin", [KV + 1, 128, BL])          # cond input^T + one-hot
    d_prev8 = inp("prev8", [SUB, 128, KX, BD], F8)  # [x0,x1,x2,x0,x1,x2]

    d_out = nc.dram_tensor("out", [BL, SEQ, V], F32, kind="ExternalOutput")

    with TileContext(nc) as tc:
        with (
            tc.sbuf_pool(name="const", bufs=1) as cp,
            tc.sbuf_pool(name="work", bufs=3) as wk,
            tc.psum_pool(name="pg", bufs=3) as pg,
            tc.psum_pool(name="pfc", bufs=1) as pfc,
        ):
            # ---------------- resident loads ----------------
            def load_const(name, dram, shape, dtype=BF, eng=nc.sync):
                t = cp.tile(shape, dtype, tag=name, name=name)
                eng.dma_start(out=t[:], in_=dram[:])
                return t

            ident8p = load_const("ident8p", d_id8p, [128, 2, 128], F8)
            fcb = load_const("fcb", d_fcb, [1, V], BF)
            ones = load_const("ones", d_ones, [1, 128], BF)
            zt = cp.tile([128, (KZ + 1) * BL], BF, tag="zt")
            for k in range(KZ + 1):
                nc.sync.dma_start(out=zt[:, k * BL:(k + 1) * BL], in_=d_zt[k])
            cin = cp.tile([128, (KV + 1) * BL], BF, tag="cin")
            for k in range(KV + 1):
                nc.sync.dma_start(out=cin[:, k * BL:(k + 1) * BL], in_=d_cin[k])

            # decoder fp8 weights: whole-tensor DMAs (amortize 500ns floor)
            w1h8 = cp.tile([128, NT, KH, 128], F8, tag="w1h8")
            w2i8 = cp.tile([128, NT, KI, 128], F8, tag="w2i8")
            w2h8 = cp.tile([128, NT, KH, 128], F8, tag="w2h8")
            w1i8 = cp.tile([128, NT, KX, 128], F8, tag="w1i8")
            fcw = cp.tile([128, KH * V], BF, tag="fcw")

            # conductor outputs consumed by the decoder
            ctb = cp.tile([128, KZ * BD], BF, tag="ctb")
            ctb8 = cp.tile([128, 6, BD], F8, tag="ctb8")   # q8(ct)+const pair
            zdec8 = cp.tile([128, NT + 1, BD], F8, tag="zdec8")
            h1f8 = cp.tile([128, KI, BD], F8, tag="h1f8")
            nc.scalar.dma_start(out=h1f8[:, 8:10, :], in_=d_const8[:])
            nc.scalar.dma_start(out=ctb8[:, 4:6, :], in_=d_const8[:])

            # ================= conductor ==================================
            with tc.sbuf_pool(name="cond", bufs=1) as cpd:
                W = KH * BL   # 512
                zcon = cpd.tile([128, NT * BL], BF, tag="zcon")
                h1c = cpd.tile([128, W], BF, tag="h1c")
                h1c8 = cpd.tile([128, KI, BL], F8, tag="h1c8")
                h2c8 = cpd.tile([128, KH, BL], F8, tag="h2c8")
                h2cb = cpd.tile([128, W + BL], BF, tag="h2cb")  # + one-hot ch
                c1c = cpd.tile([128, W], BF, tag="c1c")
                c2c = cpd.tile([128, W], BF, tag="c2c")
                cgates = cpd.tile([128, 4 * W], BF, tag="cgates")
                ctmp = cpd.tile([128, W], BF, tag="ctmp")
                cth = cpd.tile([128, W], BF, tag="cth")
                ident = cpd.tile([128, 128], BF, tag="ident")
                nc.sync.dma_start(out=ident[:], in_=d_id[:])

                for t in (c1c, c2c, h2cb):
                    nc.vector.memset(t[:], 0.0)
                nc.vector.memset(h2cb[0:1, W:W + BL], 1.0)
                nc.gpsimd.tensor_copy(
                    h2c8[:].rearrange('p a b -> p (a b)'), h2cb[:, 0:W])
                nc.scalar.dma_start(out=h1c8[:, 8:10, :],
                                    in_=d_const8[:, :, 0:BL])

                # zcon = cw1i^T @ cin + cb1 (bias via const chunk 3), banked
                for ty in range(4):
                    w = cpd.tile([128, KH, 4, 128], BF, tag="s_cw1i", bufs=2)
                    nc.gpsimd.dma_start(out=w[:],
                                        in_=d_cw1i[:, KH * ty:KH * ty + KH])
                    pt = pg.tile([128, W], F32, tag="ps")
                    for j in range(KH):
                        sl = pt[:, j * BL:(j + 1) * BL]
                        for k in range(KV + 1):
                            nc.tensor.matmul(
                                sl, w[:, j, k, :],
                                cin[:, k * BL:(k + 1) * BL],
                                start=(j == 0 and k == 0),
                                stop=(j == KH - 1 and k == KV),
                                skip_group_check=True)
                    nc.scalar.activation(zcon[:, ty * W:(ty + 1) * W], pt[:],
                                         AF.Copy)

                # h1c = tanh(ciw^T @ zt + cib) (bias via const chunk 4)
                ciw = cpd.tile([128, KH, 5, 128], BF, tag="ciw")
                nc.gpsimd.dma_start(out=ciw[:], in_=d_ciw[:])
                pt = pg.tile([128, W], F32, tag="ps")
                for m in range(KH):
                    sl = pt[:, m * BL:(m + 1) * BL]
                    for k in range(KZ + 1):
                        nc.tensor.matmul(
                            sl, ciw[:, m, k, :], zt[:, k * BL:(k + 1) * BL],
                            start=(m == 0 and k == 0),
                            stop=(m == KH - 1 and k == KZ),
                            skip_group_check=True)
                nc.scalar.activation(h1c[:], pt[:], AF.Tanh)
                nc.vector.tensor_copy(
                    h1c8[:, 0:KH, :].rearrange('p a b -> p (a b)'), h1c[:])

                for step in range(nsteps_cond):
                    # ---- L1 (bf16) ----
                    for ty in range(4):
                        pt = pg.tile([128, W], F32, tag="ps")
                        for j in range(KH):
                            m = KH * ty + j
                            sl = pt[:, j * BL:(j + 1) * BL]
                            nc.tensor.matmul(
                                sl, ident[:], zcon[:, m * BL:(m + 1) * BL],
                                start=(j == 0), stop=False,
                                skip_group_check=True)
                            w = cpd.tile([128, CH], BF, tag="s_cw1h", bufs=3)
                            nc.gpsimd.dma_start(out=w[:], in_=d_cw1h[m])
                            for k in range(KH):
                                nc.tensor.matmul(
                                    sl, w[:, k * 128:(k + 1) * 128],
                                    h1c[:, k * BL:(k + 1) * BL],
                                    start=False,
                                    stop=(j == KH - 1 and k == KH - 1),
                                    skip_group_check=True)
                        func = AF.Tanh if ty == 2 else AF.Sigmoid
                        nc.scalar.activation(cgates[:, ty * W:(ty + 1) * W],
                                             pt[:], func)
                    gi, gf = cgates[:, 0:W], cgates[:, W:2 * W]
                    gg, go = cgates[:, 2 * W:3 * W], cgates[:, 3 * W:4 * W]
                    nc.vector.tensor_mul(ctmp[:], gi, gg)
                    nc.gpsimd.tensor_mul(c1c[:], c1c[:], gf)
                    nc.gpsimd.tensor_add(c1c[:], c1c[:], ctmp[:])
                    nc.scalar.activation(cth[:], c1c[:], AF.Tanh)
                    nc.vector.tensor_mul(h1c[:], go, cth[:])
                    nc.gpsimd.tensor_copy(
                        h1c8[:, 0:KH, :].rearrange('p a b -> p (a b)'),
                        h1c[:])

                    # ---- L2 (fp8 DoubleRow) ----
                    for ty in range(4):
                        pt = pg.tile([128, W], F32, tag="ps")
                        whs = []
                        for j in range(KH):
                            m = KH * ty + j
                            sl = pt[:, j * BL:(j + 1) * BL]
                            wh = cpd.tile([128, KH, 128], F8, tag="s_cw2h8",
                                          bufs=3)
                            nc.scalar.dma_start(out=wh[:], in_=d_cw2h8[m])
                            whs.append(wh)
                            for t in range(KH // 2):
                                nc.tensor.matmul(
                                    sl, wh[:, 2 * t:2 * t + 2, :],
                                    h2c8[:, 2 * t:2 * t + 2, :],
                                    start=(j == 0 and t == 0), stop=False,
                                    perf_mode=PM, skip_group_check=True)
                        for j in range(KH):
                            m = KH * ty + j
                            sl = pt[:, j * BL:(j + 1) * BL]
                            wi = cpd.tile([128, KI, 128], F8, tag="s_cw2i8",
                                          bufs=3)
                            nc.sync.dma_start(out=wi[:], in_=d_cw2i8[m])
                            for t in range(KI // 2):
                                nc.tensor.matmul(
                                    sl, wi[:, 2 * t:2 * t + 2, :],
                                    h1c8[:, 2 * t:2 * t + 2, :],
                                    start=False,
                                    stop=(j == KH - 1 and t == KI // 2 - 1),
                                    perf_mode=PM, skip_group_check=True)
                        func = AF.Tanh if ty == 2 else AF.Sigmoid
                        nc.scalar.activation(cgates[:, ty * W:(ty + 1) * W],
                                             pt[:], func)
                    nc.vector.tensor_mul(ctmp[:], gi, gg)
                    nc.gpsimd.tensor_mul(c2c[:], c2c[:], gf)
                    nc.gpsimd.tensor_add(c2c[:], c2c[:], ctmp[:])
                    nc.scalar.activation(cth[:], c2c[:], AF.Tanh)
                    nc.vector.tensor_mul(h2cb[:, 0:W], go, cth[:])
                    nc.gpsimd.tensor_copy(
                        h2c8[:].rearrange('p a b -> p (a b)'), h2cb[:, 0:W])

                    # ---- c head -> ctb bf16 (cob via const chunk 8) ----
                    pt = pg.tile([128, KZ * BL], F32, tag="ps")
                    for m in range(KZ):
                        w = cpd.tile([128, KH + 1, 128], BF, tag="s_cow",
                                     bufs=2)
                        nc.gpsimd.dma_start(out=w[:], in_=d_cow[m])
                        sl = pt[:, m * BL:(m + 1) * BL]
                        for k in range(KH + 1):
                            nc.tensor.matmul(
                                sl, w[:, k, :],
                                h2cb[:, k * BL:(k + 1) * BL],
                                start=(m == 0 and k == 0),
                                stop=(m == KZ - 1 and k == KH),
                                skip_group_check=True)
                    ctv = ctb[:].rearrange('p (m w) -> p m w', m=KZ)[
                        :, :, step * BL:(step + 1) * BL]
                    nc.scalar.activation(ctv, pt[:], AF.Copy)

                    # decoder weight loads overlap conductor/epilogue
                    if step == 0:
                        nc.sync.dma_start(out=w1h8[:], in_=d_w1h8[:])
                    else:
                        nc.sync.dma_start(out=w1i8[:], in_=d_w1i8[:])

            nc.sync.dma_start(out=w2h8[:], in_=d_w2h8[:])
            nc.sync.dma_start(out=w2i8[:], in_=d_w2i8[:])
            for k in range(KH):
                nc.sync.dma_start(out=fcw[:, k * V:(k + 1) * V], in_=d_fcw[k])

            # ================= decoder ====================================
            with tc.sbuf_pool(name="dec", bufs=1) as dp:
                h2f8 = dp.tile([128, KH, BD], F8, tag="h2f8")
                h2bf = dp.tile([128, KH * BD], BF, tag="h2bf")
                c1 = dp.tile([128, KH * BD], BF, tag="c1")
                c2 = dp.tile([128, KH * BD], BF, tag="c2")
                gates1 = dp.tile([128, NT * BD], BF, tag="gates1")
                gates2 = dp.tile([128, NT * BD], BF, tag="gates2")
                tmp = dp.tile([128, KH * BD], BF, tag="tmp")
                th1 = dp.tile([128, KH * BD], BF, tag="th1")
                th2 = dp.tile([128, KH * BD], BF, tag="th2")
                lgh = dp.tile([128, FCB * V], BF, tag="lgh")

                for t in (c1, c2, th2):
                    nc.vector.memset(t[:], 0.0)
                nc.gpsimd.tensor_copy(
                    h2f8[:].rearrange('p a b -> p (a b)'), th2[:])
                nc.gpsimd.tensor_copy(zdec8[:, NT, :], th2[:, 0:BD])

                # epilogue of conductor (fp8 DoubleRow vs ctb8):
                # ctb8 chunks 0-3 = q8(ctb); 4/5 = const pair (loaded above)
                nc.vector.tensor_copy(
                    ctb8[:, 0:4, :].rearrange('p a b -> p (a b)'), ctb[:])
                # h1_0 = tanh(diw^T @ ct + dib) -> h1f8 fp8 direct
                diw8 = dp.tile([128, KH, 6, 128], F8, tag="diw8")
                nc.scalar.dma_start(out=diw8[:], in_=d_diw8[:])
                pt = pg.tile([128, KH * BD], F32, tag="ps")
                for m in range(KH):
                    sl = pt[:, m * BD:(m + 1) * BD]
                    for t in range(3):
                        nc.tensor.matmul(
                            sl, diw8[:, m, 2 * t:2 * t + 2, :],
                            ctb8[:, 2 * t:2 * t + 2, :],
                            start=(m % 4 == 0 and t == 0),
                            stop=(m == KH - 1 and t == 2),
                            perf_mode=PM, skip_group_check=True)
                nc.scalar.activation(
                    h1f8[:, 0:KH, :].rearrange('p a b -> p (a b)'), pt[:],
                    AF.Tanh)
                # zdec8[m] = w1c[m]^T @ ct + db1[m] (fp8 direct, banked)
                for ty in range(4):
                    wp = dp.tile([128, KH, 6, 128], F8, tag="s_w1c8",
                                 bufs=2)
                    nc.scalar.dma_start(out=wp[:],
                                        in_=d_w1c8[:, 8 * ty:8 * ty + 8])
                    pt = pg.tile([128, KH * BD], F32, tag="ps")
                    for j in range(KH):
                        sl = pt[:, j * BD:(j + 1) * BD]
                        for t in range(3):
                            nc.tensor.matmul(
                                sl, wp[:, j, 2 * t:2 * t + 2, :],
                                ctb8[:, 2 * t:2 * t + 2, :],
                                start=(j % 4 == 0 and t == 0),
                                stop=(j == KH - 1 and t == 2),
                                perf_mode=PM, skip_group_check=True)
                    nc.scalar.activation(
                        zdec8[:, 8 * ty:8 * ty + 8, :].rearrange(
                            'p a b -> p (a b)'), pt[:], AF.Copy)

                def emit_fc(s):
                    ps = pfc.tile([128, V], F32, tag="pfc")
                    nc.tensor.matmul(ps[:], ones[:, :128], fcb[:], start=True,
                                     stop=False)
                    for k in range(KH):
                        nc.tensor.matmul(ps[:], h2bf[:, k * BD:(k + 1) * BD],
                                         fcw[:, k * V:(k + 1) * V],
                                         start=False, stop=(k == KH - 1))
                    b = s % FCB
                    nc.scalar.activation(lgh[:, b * V:(b + 1) * V], ps[:],
                                         AF.Copy)

                def flush_softmax(s_lo, s_hi):
                    for s in range(s_lo, s_hi + 1):
                        b = s % FCB
                        lg = lgh[:, b * V:(b + 1) * V]
                        nmx = wk.tile([128, 1], F32, tag="nmx")
                        nc.vector.reduce_max(nmx[:], lg,
                                             axis=mybir.AxisListType.X,
                                             negate=True)
                        ex = wk.tile([128, V], F32, tag="ex")
                        se = wk.tile([128, 1], F32, tag="se")
                        nc.scalar.activation(ex[:], lg, AF.Exp, bias=nmx[:],
                                             accum_out=se[:])
                        lse = wk.tile([128, 1], F32, tag="lse")
                        nc.scalar.activation(lse[:], se[:], AF.Ln)
                        nc.vector.tensor_sub(lse[:], lse[:], nmx[:])
                        ot = wk.tile([128, V], F32, tag="ot")
                        nc.vector.tensor_scalar_sub(ot[:], lg, lse[:])
                        for u in range(U):
                            nc.sync.dma_start(
                                out=d_out[0:BL, u * SUB + s, :],
                                in_=ot[u * BL:(u + 1) * BL, :])

                for s in range(nsteps_dec):
                    xt = dp.tile([128, KX, BD], F8, tag="xt", bufs=3)
                    nc.sync.dma_start(out=xt[:], in_=d_prev8[s])

                    # ---- L1: one [128,1024] psum tile per gate type ----
                    for ty in range(4):
                        pt = pg.tile([128, 1024], F32, tag="ps")
                        for j in range(KH):
                            m = KH * ty + j
                            sl = pt[:, j * BD:(j + 1) * BD]
                            nc.tensor.matmul(
                                sl, ident8p[:], zdec8[:, m:m + 2, :],
                                start=(j in (0, 4)), stop=False,
                                perf_mode=PM, skip_group_check=True)
                            for t in range(KX // 2):
                                nc.tensor.matmul(
                                    sl, w1i8[:, m, 2 * t:2 * t + 2, :],
                                    xt[:, 2 * t:2 * t + 2, :],
                                    start=False, stop=False, perf_mode=PM,
                                    skip_group_check=True)
                            for t in range(KH // 2):
                                nc.tensor.matmul(
                                    sl, w1h8[:, m, 2 * t:2 * t + 2, :],
                                    h1f8[:, 2 * t:2 * t + 2, :],
                                    start=False,
                                    stop=(j == KH - 1 and t == 3),
                                    perf_mode=PM, skip_group_check=True)
                        func = AF.Tanh if ty == 2 else AF.Sigmoid
                        nc.scalar.activation(
                            gates1[:, ty * 1024:(ty + 1) * 1024], pt[:], func)

                    # ---- L1 cell (2 chunk groups) ----
                    for q in range(2):
                        w = slice(q * 512, (q + 1) * 512)
                        gi = gates1[:, q * 512:(q + 1) * 512]
                        gf = gates1[:, 1024 + q * 512:1024 + (q + 1) * 512]
                        gg = gates1[:, 2048 + q * 512:2048 + (q + 1) * 512]
                        go = gates1[:, 3072 + q * 512:3072 + (q + 1) * 512]
                        nc.vector.tensor_mul(tmp[:, w], gi, gg)
                        nc.gpsimd.tensor_mul(c1[:, w], c1[:, w], gf)
                        nc.gpsimd.tensor_add(c1[:, w], c1[:, w], tmp[:, w])
                        nc.scalar.activation(th1[:, w], c1[:, w], AF.Tanh)
                        nc.vector.tensor_mul(
                            h1f8[:, 4 * q:4 * (q + 1), :].rearrange(
                                'p a b -> p (a b)'),
                            go, th1[:, w])

                    if s > 0:
                        emit_fc(s - 1)

                    # ---- L2 ----
                    for ty in range(4):
                        pt = pg.tile([128, 1024], F32, tag="ps")
                        for j in range(KH):
                            m = KH * ty + j
                            sl = pt[:, j * BD:(j + 1) * BD]
                            for t in range(KH // 2):   # old state first
                                nc.tensor.matmul(
                                    sl, w2h8[:, m, 2 * t:2 * t + 2, :],
                                    h2f8[:, 2 * t:2 * t + 2, :],
                                    start=(j in (0, 4) and t == 0),
                                    stop=False,
                                    perf_mode=PM, skip_group_check=True)
                        for j in range(KH):
                            m = KH * ty + j
                            sl = pt[:, j * BD:(j + 1) * BD]
                            for t in range(KI // 2):   # h1 + const(db2)
                                nc.tensor.matmul(
                                    sl, w2i8[:, m, 2 * t:2 * t + 2, :],
                                    h1f8[:, 2 * t:2 * t + 2, :],
                                    start=False,
                                    stop=(j == KH - 1 and t == KI // 2 - 1),
                                    perf_mode=PM, skip_group_check=True)
                        func = AF.Tanh if ty == 2 else AF.Sigmoid
                        nc.scalar.activation(
                            gates2[:, ty * 1024:(ty + 1) * 1024], pt[:], func)

                    # ---- L2 cell (2 chunk groups) ----
                    for q in range(2):
                        w = slice(q * 512, (q + 1) * 512)
                        gi = gates2[:, q * 512:(q + 1) * 512]
                        gf = gates2[:, 1024 + q * 512:1024 + (q + 1) * 512]
                        gg = gates2[:, 2048 + q * 512:2048 + (q + 1) * 512]
                        go = gates2[:, 3072 + q * 512:3072 + (q + 1) * 512]
                        nc.vector.tensor_mul(tmp[:, w], gi, gg)
                        nc.gpsimd.tensor_mul(c2[:, w], c2[:, w], gf)
                        nc.gpsimd.tensor_add(c2[:, w], c2[:, w], tmp[:, w])
                        nc.scalar.activation(th2[:, w], c2[:, w], AF.Tanh)
                        nc.vector.tensor_mul(h2bf[:, w], go, th2[:, w])
                        nc.gpsimd.tensor_copy(
                            h2f8[:, 4 * q:4 * (q + 1), :].rearrange(
                                'p a b -> p (a b)'), h2bf[:, w])

                    if s % FCB == FCB - 1:
                        emit_fc(s)
                        flush_softmax(s - FCB + 1, s)

    nc.compile()
    return nc


# --------------------------------------------------------------------------
# Host-side packing
# --------------------------------------------------------------------------

def _mmajor(w, kin_pad=None):
    kin, g = w.shape
    kp = kin_pad or kin
    nt = g // 128
    wp = np.zeros((kp, g), dtype=np.float32)
    wp[:kin] = w
    out = (wp.reshape(kp // 128, 128, nt, 128)
             .transpose(2, 1, 0, 3)
             .reshape(nt, 128, kp))
    return np.ascontiguousarray(out.astype(bf16))


def _mmajor8(wp, kc):
    g = wp.shape[1]
    nt = g // 128
    out = wp.reshape(kc, 128, nt, 128).transpose(2, 1, 0, 3)
    return np.ascontiguousarray(out.astype(f8))


def _pad_rows(w, rows):
    out = np.zeros((rows, w.shape[1]), dtype=np.float32)
    out[:w.shape[0]] = w
    return out


def _kpack(b):
    return np.ascontiguousarray(b.reshape(-1, 128).T.astype(np.float32))


def _mmajor_b(w, kc, bias=None, dtype=None):
    """[Kin, G] f32 -> [G//128, 128, kc, 128]; optional bias row at
    chunk kc-2 (fp8 pairs) or the last chunk."""
    kin, g = w.shape
    nt = g // 128
    wp = np.zeros((kc * 128, g), dtype=np.float32)
    wp[:kin] = w
    if bias is not None:
        wp[(kc - 2 if (dtype or bf16) == f8 else kc - 1) * 128] = bias
    out = wp.reshape(kc, 128, nt, 128).transpose(2, 1, 0, 3)
    return np.ascontiguousarray(out.astype(dtype or bf16))


def pack_inputs(i):
    m = {}
    pm = lambda a: np.ascontiguousarray(a.transpose(1, 0, 2, 3))
    m["w1h8"] = pm(_mmajor8(_pad_rows(i["dW1h"], KH * 128), KH))
    m["w2h8"] = pm(_mmajor8(_pad_rows(i["dW2h"], KH * 128), KH))
    w2i = _pad_rows(i["dW2i"], KI * 128)
    w2i[8 * 128] = i["db2i"] + i["db2h"]
    m["w2i8"] = pm(_mmajor8(w2i, KI))
    w1x = i["dW1i"][CO:]
    w1x8 = w1x.astype(f8).astype(np.float32)
    w1xr = w1x - w1x8
    wx = np.zeros((KX * 128, G), dtype=np.float32)
    wx[:V] = w1x8
    wx[VP:VP + V] = w1xr
    m["w1i8"] = pm(_mmajor8(wx, KX))
    m["w1c8"] = pm(_mmajor_b(i["dW1i"][:CO], 6,
                             i["db1i"] + i["db1h"], f8))
    c8 = np.zeros((128, 2, BD), dtype=np.float32)
    c8[0, 0, :] = 1.0
    m["const8"] = c8.astype(f8)
    id8 = np.zeros((128, 2, 128), dtype=np.float32)
    id8[:, 0, :] = np.eye(128, dtype=np.float32)
    m["ident8p"] = id8.astype(f8)
    m["cw1h"] = _mmajor(i["cW1h"])
    cw2i = _pad_rows(i["cW2i"], KI * 128)
    cw2i[8 * 128] = i["cb2i"] + i["cb2h"]
    m["cw2i8"] = _mmajor8(cw2i, KI)
    m["cw2h8"] = _mmajor8(_pad_rows(i["cW2h"], KH * 128), KH)
    m["cw1i"] = np.ascontiguousarray(_mmajor_b(
        i["cW1i"], 4, i["cb1i"] + i["cb1h"]).transpose(1, 0, 2, 3))
    m["ciw"] = np.ascontiguousarray(_mmajor_b(
        i["ci_W"], KZ + 1, i["ci_b"]).transpose(1, 0, 2, 3))
    m["cow"] = _mmajor_b(i["co_W"], KH + 1, i["co_b"])
    m["diw8"] = np.ascontiguousarray(_mmajor_b(
        i["di_W"], 6, i["di_b"], f8).transpose(1, 0, 2, 3))
    m["fcw"] = np.ascontiguousarray(
        i["fc_W"].reshape(KH, 128, V)).astype(bf16)
    m["fcb"] = i["fc_b"].reshape(1, V).astype(bf16)
    m["ones"] = np.ones((1, 128), dtype=bf16)
    m["ident"] = np.eye(128, dtype=bf16)
    return m


def pack_core(i, core):
    s = slice(core * BL, (core + 1) * BL)
    m = {}
    zt = np.zeros(((KZ + 1) * 128, BL), dtype=np.float32)
    zt[:Z] = i["z"][s].T
    zt[Z] = 1.0
    m["zt"] = np.ascontiguousarray(zt.reshape(KZ + 1, 128, BL).astype(bf16))
    ci = np.zeros(((KV + 1) * 128, BL), dtype=np.float32)
    ci[:V] = i["conductor_input"][0, s].T
    ci[VP] = 1.0
    m["cin"] = np.ascontiguousarray(ci.reshape(KV + 1, 128, BL).astype(bf16))
    x = i["x"]
    prev = np.zeros((SUB, VP, U, BL), dtype=np.float32)
    for t in range(SUB):
        for u in range(U):
            sq = u * SUB + t
            if sq == 0:
                prev[t, 0, u, :] = 1.0
            else:
                prev[t, :V, u, :] = x[sq - 1, s].T
    pc = prev.reshape(SUB, KV, 128, U * BL)
    m["prev8"] = np.ascontiguousarray(
        np.concatenate([pc, pc], axis=1).transpose(0, 2, 1, 3).astype(f8))
    return m


_NC_CACHE = {}


def _get_nc(key=(U, SUB)):
    if key not in _NC_CACHE:
        _NC_CACHE[key] = build_nc(*key)
    return _NC_CACHE[key]


def kernel(**inputs):
    inputs = {k: np.asarray(v) for k, v in inputs.items()}
    nc = _get_nc()
    shared = pack_inputs(inputs)
    in_maps = [dict(shared, **pack_core(inputs, c)) for c in range(NCORES)]
    r = run_bass_kernel_spmd(nc, in_maps, core_ids=list(range(NCORES)))
    out = np.concatenate([r.results[c]["out"] for c in range(NCORES)], axis=0)
    return out.astype(np.float32)


# revision 3
# speedup vs baseline: 1.0373x; 1.0373x over previous
"""Trainium2 Bass kernel for nn_Decoder_29678224015654 — fp8 DoubleRow, v3.

v3 over v2:
- Conductor restructured: banked PSUM (one [128,512] bank per gate type,
  8 m-tiles of BL=64), banked activations, whole-layer cell math on
  DVE/Pool. L1 stays bf16 (zcon identity-injection carries cb1); L2 runs
  fp8 DoubleRow (cw2i/cw2h fp8 streams, cb2 via fp8 const chunk pair).
- Decoder gate PSUM tiles widened to [128,1024] (one per gate type,
  start_tensor_calc at both 2KB zero-regions), single activation per
  gate type and single tanh(c) per layer.
- zdec injected via fp8 ident-pair DoubleRow matmul (64 cyc vs 128).
- log_softmax batched: per step only the fc matmul + a Copy of logits to
  a bf16 history; the Exp/Ln chains run every 8 steps so the activation
  table (sigmoid/tanh) is not reloaded twice per step (1283 ns each).
- DMA queues balanced: SP carries decoder fp8 weights, gpsimd the
  conductor L1 streams, Activation the conductor L2 fp8 streams.
"""
import sys
sys.path.insert(0, '/opt/trn_rl_repo')

import numpy as np
import ml_dtypes

import concourse.bass as bass
import concourse.mybir as mybir
from concourse import bacc
from concourse.tile import TileContext
from concourse.bass_utils import run_bass_kernel_spmd

AF = mybir.ActivationFunctionType
BF = mybir.dt.bfloat16
F32 = mybir.dt.float32
F8 = mybir.dt.float8e4
PM = mybir.MatmulPerfMode.DoubleRow

SEQ, SUB, B, V = 32, 16, 512, 258
U = SEQ // SUB              # 2
Z, CH, CO, DH = 512, 1024, 512, 1024
NCORES = 8
BL = B // NCORES            # 64 batch rows per core
BD = BL * U                 # 128 decoder effective batch
VP = 384                    # V padded to 3 K-chunks
G = 4 * DH                  # 4096 gate cols
NT = G // 128               # 32 gate tiles
KH = DH // 128              # 8 K-chunks for hidden=1024
KZ = Z // 128               # 4
KV = VP // 128              # 3
KX = 6                      # W1x chunks: [W1x8(3), W1xr8(3)]
KI = 10                     # L2 h1 chunks: 8 + const pair (bias)
FCB = 8                     # softmax flush batch

bf16 = ml_dtypes.bfloat16
f8 = ml_dtypes.float8_e4m3


def build_nc(nsteps_cond=U, nsteps_dec=SUB, num_devices=NCORES):
    nc = bacc.Bacc("TRN2", target_bir_lowering=False, debug=False,
                   num_devices=num_devices)

    def inp(name, shape, dtype=BF):
        return nc.dram_tensor(name, list(shape), dtype, kind="ExternalInput")

    # fp8 decoder weights, m-tile-major, k-chunk pairs for DoubleRow
    d_w1h8 = inp("w1h8", [128, NT, KH, 128], F8)
    d_w2i8 = inp("w2i8", [128, NT, KH, 128], F8)
    d_w2h8 = inp("w2h8", [128, NT, KH, 128], F8)
    d_w1i8 = inp("w1i8", [128, NT, KX, 128], F8)   # [W1x8, W1xr8]
    d_w1c8 = inp("w1c8", [128, NT, 6, 128], F8)    # chunk 4 = db1 row
    d_const8 = inp("const8", [128, 2, BD], F8)     # const pair (p0 = 1.0)
    d_db2b8 = inp("db2b8", [128, NT + 1, BD], F8)  # db2 bcast pairs
    d_cb2b8 = inp("cb2b8", [128, NT + 1, BL], F8)  # cb2 bcast pairs
    # conductor: L1 bf16, L2 fp8
    d_cw1h = inp("cw1h", [NT, 128, CH])
    d_cw2i8 = inp("cw2i8", [128, NT, KH, 128], F8)
    d_cw2h8p = inp("cw2h8p", [128, NT, KH, 128], F8)
    d_cw1i = inp("cw1i", [128, NT, 4, 128])        # bf16; chunk 3 = cb1 row
    d_ciw = inp("ciw", [128, KH, 5, 128])          # bf16; chunk 4 = cib row
    d_cow = inp("cow", [KZ, 128, KH + 1, 128])     # bf16; chunk 8 = cob row
    d_diw8 = inp("diw8", [128, KH, 6, 128], F8)    # chunk 4 = dib row
    d_fcw = inp("fcw", [KH, 128, V])

    d_id = inp("ident", [128, 128])                # bf16 identity
    d_id8p = inp("ident8p", [128, 2, 128], F8)     # [I, 0] fp8 pair
    d_fcb = inp("fcb", [1, V])                     # bf16
    d_ones = inp("ones", [1, 128])                 # bf16

    d_zt = inp("zt", [KZ + 1, 128, BL])            # z^T + one-hot chunk
    d_cin = inp("cin", [KV + 1, 128, BL])          # cond input^T + one-hot
    d_prev8 = inp("prev8", [SUB, 128, KX, BD], F8)  # [x0,x1,x2,x0,x1,x2]

    d_out = nc.dram_tensor("out", [BL, SEQ, V], F32, kind="ExternalOutput")

    with TileContext(nc) as tc:
        with (
            tc.sbuf_pool(name="const", bufs=1) as cp,
            tc.sbuf_pool(name="work", bufs=3) as wk,
            tc.psum_pool(name="pg", bufs=3) as pg,
            tc.psum_pool(name="pfc", bufs=1) as pfc,
        ):
            # ---------------- resident loads ----------------
            def load_const(name, dram, shape, dtype=BF, eng=nc.sync):
                t = cp.tile(shape, dtype, tag=name, name=name)
                eng.dma_start(out=t[:], in_=dram[:])
                return t

            ident8p = load_const("ident8p", d_id8p, [128, 2, 128], F8)
            fcb = load_const("fcb", d_fcb, [1, V], BF)
            ones = load_const("ones", d_ones, [1, 128], BF)
            zt = cp.tile([128, (KZ + 1) * BL], BF, tag="zt")
            for k in range(KZ + 1):
                nc.sync.dma_start(out=zt[:, k * BL:(k + 1) * BL], in_=d_zt[k])
            cin = cp.tile([128, (KV + 1) * BL], BF, tag="cin")
            for k in range(KV + 1):
                nc.sync.dma_start(out=cin[:, k * BL:(k + 1) * BL], in_=d_cin[k])

            # decoder fp8 weights: whole-tensor DMAs (amortize 500ns floor)
            w1h8 = cp.tile([128, NT, KH, 128], F8, tag="w1h8")
            w2i8 = cp.tile([128, NT, KH, 128], F8, tag="w2i8")
            w2h8 = cp.tile([128, NT, KH, 128], F8, tag="w2h8")
            w1i8 = cp.tile([128, NT, KX, 128], F8, tag="w1i8")
            fcw = cp.tile([128, KH * V], BF, tag="fcw")

            # conductor outputs consumed by the decoder
            ctb = cp.tile([128, KZ * BD], BF, tag="ctb")
            ctb8 = cp.tile([128, 6, BD], F8, tag="ctb8")   # q8(ct)+const pair
            zdec8 = cp.tile([128, NT + 1, BD], F8, tag="zdec8")
            h1f8 = cp.tile([128, KH, BD], F8, tag="h1f8")
            db2b8 = cp.tile([128, NT + 1, BD], F8, tag="db2b8")
            nc.scalar.dma_start(out=db2b8[:], in_=d_db2b8[:])
            nc.scalar.dma_start(out=ctb8[:, 4:6, :], in_=d_const8[:])

            # ================= conductor ==================================
            with tc.sbuf_pool(name="cond", bufs=1) as cpd:
                W = KH * BL   # 512
                zcon = cpd.tile([128, NT * BL], BF, tag="zcon")
                h1c = cpd.tile([128, W], BF, tag="h1c")
                h1c8 = cpd.tile([128, KH, BL], F8, tag="h1c8")
                cb2b8 = cpd.tile([128, NT + 1, BL], F8, tag="cb2b8")
                nc.scalar.dma_start(out=cb2b8[:], in_=d_cb2b8[:])
                h2c8 = cpd.tile([128, KH, BL], F8, tag="h2c8")
                h2cb = cpd.tile([128, W + BL], BF, tag="h2cb")  # + one-hot ch
                c1c = cpd.tile([128, W], BF, tag="c1c")
                c2c = cpd.tile([128, W], BF, tag="c2c")
                cgates = cpd.tile([128, 4 * W], BF, tag="cgates")
                ctmp = cpd.tile([128, W], BF, tag="ctmp")
                cth = cpd.tile([128, W], BF, tag="cth")
                ident = cpd.tile([128, 128], BF, tag="ident")
                nc.sync.dma_start(out=ident[:], in_=d_id[:])

                for t in (c1c, c2c, h2cb):
                    nc.vector.memset(t[:], 0.0)
                nc.vector.memset(h2cb[0:1, W:W + BL], 1.0)
                nc.gpsimd.tensor_copy(
                    h2c8[:].rearrange('p a b -> p (a b)'), h2cb[:, 0:W])

                # zcon = cw1i^T @ cin + cb1 (bias via const chunk 3), banked
                for ty in range(4):
                    w = cpd.tile([128, KH, 4, 128], BF, tag="s_cw1i", bufs=2)
                    nc.gpsimd.dma_start(out=w[:],
                                        in_=d_cw1i[:, KH * ty:KH * ty + KH])
                    pt = pg.tile([128, W], F32, tag="ps")
                    for j in range(KH):
                        sl = pt[:, j * BL:(j + 1) * BL]
                        for k in range(KV + 1):
                            nc.tensor.matmul(
                                sl, w[:, j, k, :],
                                cin[:, k * BL:(k + 1) * BL],
                                start=(j == 0 and k == 0),
                                stop=(j == KH - 1 and k == KV),
                                skip_group_check=True)
                    nc.scalar.activation(zcon[:, ty * W:(ty + 1) * W], pt[:],
                                         AF.Copy)

                # h1c = tanh(ciw^T @ zt + cib) (bias via const chunk 4)
                ciw = cpd.tile([128, KH, 5, 128], BF, tag="ciw")
                nc.gpsimd.dma_start(out=ciw[:], in_=d_ciw[:])
                pt = pg.tile([128, W], F32, tag="ps")
                for m in range(KH):
                    sl = pt[:, m * BL:(m + 1) * BL]
                    for k in range(KZ + 1):
                        nc.tensor.matmul(
                            sl, ciw[:, m, k, :], zt[:, k * BL:(k + 1) * BL],
                            start=(m == 0 and k == 0),
                            stop=(m == KH - 1 and k == KZ),
                            skip_group_check=True)
                nc.scalar.activation(h1c[:], pt[:], AF.Tanh)
                nc.vector.tensor_copy(
                    h1c8[:, 0:KH, :].rearrange('p a b -> p (a b)'), h1c[:])

                for step in range(nsteps_cond):
                    # ---- L1 (bf16) ----
                    for ty in range(4):
                        pt = pg.tile([128, W], F32, tag="ps")
                        for j in range(KH):
                            m = KH * ty + j
                            sl = pt[:, j * BL:(j + 1) * BL]
                            nc.tensor.matmul(
                                sl, ident[:], zcon[:, m * BL:(m + 1) * BL],
                                start=(j == 0), stop=False,
                                skip_group_check=True)
                            w = cpd.tile([128, CH], BF, tag="s_cw1h", bufs=3)
                            nc.gpsimd.dma_start(out=w[:], in_=d_cw1h[m])
                            for k in range(KH):
                                nc.tensor.matmul(
                                    sl, w[:, k * 128:(k + 1) * 128],
                                    h1c[:, k * BL:(k + 1) * BL],
                                    start=False,
                                    stop=(j == KH - 1 and k == KH - 1),
                                    skip_group_check=True)
                        func = AF.Tanh if ty == 2 else AF.Sigmoid
                        nc.scalar.activation(cgates[:, ty * W:(ty + 1) * W],
                                             pt[:], func)
                    gi, gf = cgates[:, 0:W], cgates[:, W:2 * W]
                    gg, go = cgates[:, 2 * W:3 * W], cgates[:, 3 * W:4 * W]
                    nc.vector.tensor_mul(ctmp[:], gi, gg)
                    nc.gpsimd.tensor_mul(c1c[:], c1c[:], gf)
                    nc.gpsimd.tensor_add(c1c[:], c1c[:], ctmp[:])
                    nc.scalar.activation(cth[:], c1c[:], AF.Tanh)
                    nc.vector.tensor_mul(h1c[:], go, cth[:])
                    nc.gpsimd.tensor_copy(
                        h1c8[:, 0:KH, :].rearrange('p a b -> p (a b)'),
                        h1c[:])

                    # ---- L2 (fp8 DoubleRow, bias via ident8p@cb2b8) ----
                    for ty in range(4):
                        pt = pg.tile([128, W], F32, tag="ps")
                        for jg in range(KH // 2):
                            wh = cpd.tile([128, 2, KH, 128], F8,
                                          tag="s_cw2h8", bufs=2)
                            m0 = KH * ty + 2 * jg
                            nc.scalar.dma_start(out=wh[:],
                                                in_=d_cw2h8p[:, m0:m0 + 2])
                            for j2 in range(2):
                                j = 2 * jg + j2
                                m = KH * ty + j
                                sl = pt[:, j * BL:(j + 1) * BL]
                                nc.tensor.matmul(
                                    sl, ident8p[:], cb2b8[:, m:m + 2, :],
                                    start=(j == 0), stop=False,
                                    perf_mode=PM, skip_group_check=True)
                                for t in range(KH // 2):
                                    nc.tensor.matmul(
                                        sl, wh[:, j2, 2 * t:2 * t + 2, :],
                                        h2c8[:, 2 * t:2 * t + 2, :],
                                        start=False, stop=False,
                                        perf_mode=PM, skip_group_check=True)
                        for jg in range(KH // 2):
                            wi = cpd.tile([128, 2, KH, 128], F8,
                                          tag="s_cw2i8", bufs=2)
                            m0 = KH * ty + 2 * jg
                            nc.sync.dma_start(out=wi[:],
                                              in_=d_cw2i8[:, m0:m0 + 2])
                            for j2 in range(2):
                                j = 2 * jg + j2
                                sl = pt[:, j * BL:(j + 1) * BL]
                                for t in range(KH // 2):
                                    nc.tensor.matmul(
                                        sl, wi[:, j2, 2 * t:2 * t + 2, :],
                                        h1c8[:, 2 * t:2 * t + 2, :],
                                        start=False,
                                        stop=(j == KH - 1 and t == 3),
                                        perf_mode=PM, skip_group_check=True)
                        func = AF.Tanh if ty == 2 else AF.Sigmoid
                        nc.scalar.activation(cgates[:, ty * W:(ty + 1) * W],
                                             pt[:], func)
                    nc.vector.tensor_mul(ctmp[:], gi, gg)
                    nc.gpsimd.tensor_mul(c2c[:], c2c[:], gf)
                    nc.gpsimd.tensor_add(c2c[:], c2c[:], ctmp[:])
                    nc.scalar.activation(cth[:], c2c[:], AF.Tanh)
                    nc.vector.tensor_mul(h2cb[:, 0:W], go, cth[:])
                    nc.gpsimd.tensor_copy(
                        h2c8[:].rearrange('p a b -> p (a b)'), h2cb[:, 0:W])

                    # ---- c head -> ctb bf16 (cob via const chunk 8) ----
                    pt = pg.tile([128, KZ * BL], F32, tag="ps")
                    for m in range(KZ):
                        w = cpd.tile([128, KH + 1, 128], BF, tag="s_cow",
                                     bufs=2)
                        nc.gpsimd.dma_start(out=w[:], in_=d_cow[m])
                        sl = pt[:, m * BL:(m + 1) * BL]
                        for k in range(KH + 1):
                            nc.tensor.matmul(
                                sl, w[:, k, :],
                                h2cb[:, k * BL:(k + 1) * BL],
                                start=(m == 0 and k == 0),
                                stop=(m == KZ - 1 and k == KH),
                                skip_group_check=True)
                    ctv = ctb[:].rearrange('p (m w) -> p m w', m=KZ)[
                        :, :, step * BL:(step + 1) * BL]
                    nc.scalar.activation(ctv, pt[:], AF.Copy)

                    # decoder weight loads overlap conductor/epilogue
                    if step == 0:
                        nc.sync.dma_start(out=w1h8[:], in_=d_w1h8[:])
                    else:
                        nc.sync.dma_start(out=w1i8[:], in_=d_w1i8[:])

            nc.sync.dma_start(out=w2h8[:], in_=d_w2h8[:])
            nc.sync.dma_start(out=w2i8[:], in_=d_w2i8[:])
            for k in range(KH):
                nc.sync.dma_start(out=fcw[:, k * V:(k + 1) * V], in_=d_fcw[k])

            # ================= decoder ====================================
            with tc.sbuf_pool(name="dec", bufs=1) as dp:
                h2f8 = dp.tile([128, KH, BD], F8, tag="h2f8")
                h2bf = dp.tile([128, KH * BD], BF, tag="h2bf")
                c1 = dp.tile([128, KH * BD], BF, tag="c1")
                c2 = dp.tile([128, KH * BD], BF, tag="c2")
                gates1 = dp.tile([128, NT * BD], BF, tag="gates1")
                gates2 = dp.tile([128, NT * BD], BF, tag="gates2")
                tmp = dp.tile([128, KH * BD], BF, tag="tmp")
                th1 = dp.tile([128, KH * BD], BF, tag="th1")
                th2 = dp.tile([128, KH * BD], BF, tag="th2")
                lgh = dp.tile([128, FCB * V], BF, tag="lgh")

                for t in (c1, c2, th2):
                    nc.vector.memset(t[:], 0.0)
                nc.gpsimd.tensor_copy(
                    h2f8[:].rearrange('p a b -> p (a b)'), th2[:])
                nc.gpsimd.tensor_copy(zdec8[:, NT, :], th2[:, 0:BD])

                # epilogue of conductor (fp8 DoubleRow vs ctb8):
                # ctb8 chunks 0-3 = q8(ctb); 4/5 = const pair (loaded above)
                nc.vector.tensor_copy(
                    ctb8[:, 0:4, :].rearrange('p a b -> p (a b)'), ctb[:])
                # h1_0 = tanh(diw^T @ ct + dib) -> h1f8 fp8 direct
                diw8 = dp.tile([128, KH, 6, 128], F8, tag="diw8")
                nc.sync.dma_start(out=diw8[:], in_=d_diw8[:])
                pt = pg.tile([128, KH * BD], F32, tag="ps")
                for m in range(KH):
                    sl = pt[:, m * BD:(m + 1) * BD]
                    for t in range(3):
                        nc.tensor.matmul(
                            sl, diw8[:, m, 2 * t:2 * t + 2, :],
                            ctb8[:, 2 * t:2 * t + 2, :],
                            start=(m % 4 == 0 and t == 0),
                            stop=(m == KH - 1 and t == 2),
                            perf_mode=PM, skip_group_check=True)
                nc.scalar.activation(
                    h1f8[:, 0:KH, :].rearrange('p a b -> p (a b)'), pt[:],
                    AF.Tanh)
                # zdec8[m] = w1c[m]^T @ ct + db1[m] (fp8 direct, banked)
                for ty in range(4):
                    wp = dp.tile([128, KH, 6, 128], F8, tag="s_w1c8",
                                 bufs=2)
                    nc.sync.dma_start(out=wp[:],
                                        in_=d_w1c8[:, 8 * ty:8 * ty + 8])
                    pt = pg.tile([128, KH * BD], F32, tag="ps")
                    for j in range(KH):
                        sl = pt[:, j * BD:(j + 1) * BD]
                        for t in range(3):
                            nc.tensor.matmul(
                                sl, wp[:, j, 2 * t:2 * t + 2, :],
                                ctb8[:, 2 * t:2 * t + 2, :],
                                start=(j % 4 == 0 and t == 0),
                                stop=(j == KH - 1 and t == 2),
                                perf_mode=PM, skip_group_check=True)
                    nc.scalar.activation(
                        zdec8[:, 8 * ty:8 * ty + 8, :].rearrange(
                            'p a b -> p (a b)'), pt[:], AF.Copy)

                def emit_fc(s):
                    ps = pfc.tile([128, V], F32, tag="pfc")
                    nc.tensor.matmul(ps[:], ones[:, :128], fcb[:], start=True,
                                     stop=False)
                    for k in range(KH):
                        nc.tensor.matmul(ps[:], h2bf[:, k * BD:(k + 1) * BD],
                                         fcw[:, k * V:(k + 1) * V],
                                         start=False, stop=(k == KH - 1))
                    b = s % FCB
                    nc.scalar.activation(lgh[:, b * V:(b + 1) * V], ps[:],
                                         AF.Copy)

                def flush_softmax(s_lo, s_hi):
                    for s in range(s_lo, s_hi + 1):
                        b = s % FCB
                        lg = lgh[:, b * V:(b + 1) * V]
                        nmx = wk.tile([128, 1], F32, tag="nmx")
                        nc.vector.reduce_max(nmx[:], lg,
                                             axis=mybir.AxisListType.X,
                                             negate=True)
                        ex = wk.tile([128, V], F32, tag="ex")
                        se = wk.tile([128, 1], F32, tag="se")
                        nc.scalar.activation(ex[:], lg, AF.Exp, bias=nmx[:],
                                             accum_out=se[:])
                        lse = wk.tile([128, 1], F32, tag="lse")
                        nc.scalar.activation(lse[:], se[:], AF.Ln)
                        nc.vector.tensor_sub(lse[:], lse[:], nmx[:])
                        ot = wk.tile([128, V], F32, tag="ot")
                        nc.vector.tensor_scalar_sub(ot[:], lg, lse[:])
                        for u in range(U):
                            nc.sync.dma_start(
                                out=d_out[0:BL, u * SUB + s, :],
                                in_=ot[u * BL:(u + 1) * BL, :])

                for s in range(nsteps_dec):
                    xt = dp.tile([128, KX, BD], F8, tag="xt", bufs=3)
                    nc.sync.dma_start(out=xt[:], in_=d_prev8[s])

                    # ---- L1: one [128,1024] psum tile per gate type ----
                    for ty in range(4):
                        pt = pg.tile([128, 1024], F32, tag="ps")
                        for j in range(KH):
                            m = KH * ty + j
                            sl = pt[:, j * BD:(j + 1) * BD]
                            nc.tensor.matmul(
                                sl, ident8p[:], zdec8[:, m:m + 2, :],
                                start=(j in (0, 4)), stop=False,
                                perf_mode=PM, skip_group_check=True)
                            for t in range(KX // 2):
                                nc.tensor.matmul(
                                    sl, w1i8[:, m, 2 * t:2 * t + 2, :],
                                    xt[:, 2 * t:2 * t + 2, :],
                                    start=False, stop=False, perf_mode=PM,
                                    skip_group_check=True)
                            for t in range(KH // 2):
                                nc.tensor.matmul(
                                    sl, w1h8[:, m, 2 * t:2 * t + 2, :],
                                    h1f8[:, 2 * t:2 * t + 2, :],
                                    start=False,
                                    stop=(j == KH - 1 and t == 3),
                                    perf_mode=PM, skip_group_check=True)
                        func = AF.Tanh if ty == 2 else AF.Sigmoid
                        nc.scalar.activation(
                            gates1[:, ty * 1024:(ty + 1) * 1024], pt[:], func)

                    # ---- L1 cell (2 chunk groups) ----
                    for q in range(2):
                        w = slice(q * 512, (q + 1) * 512)
                        gi = gates1[:, q * 512:(q + 1) * 512]
                        gf = gates1[:, 1024 + q * 512:1024 + (q + 1) * 512]
                        gg = gates1[:, 2048 + q * 512:2048 + (q + 1) * 512]
                        go = gates1[:, 3072 + q * 512:3072 + (q + 1) * 512]
                        nc.vector.tensor_mul(tmp[:, w], gi, gg)
                        nc.gpsimd.tensor_mul(c1[:, w], c1[:, w], gf)
                        nc.gpsimd.tensor_add(c1[:, w], c1[:, w], tmp[:, w])
                        nc.scalar.activation(th1[:, w], c1[:, w], AF.Tanh)
                        nc.vector.tensor_mul(
                            h1f8[:, 4 * q:4 * (q + 1), :].rearrange(
                                'p a b -> p (a b)'),
                            go, th1[:, w])

                    if s > 0:
                        emit_fc(s - 1)

                    # ---- L2 ----
                    for ty in range(4):
                        pt = pg.tile([128, 1024], F32, tag="ps")
                        for j in range(KH):
                            m = KH * ty + j
                            sl = pt[:, j * BD:(j + 1) * BD]
                            for t in range(KH // 2):   # old state first
                                nc.tensor.matmul(
                                    sl, w2h8[:, m, 2 * t:2 * t + 2, :],
                                    h2f8[:, 2 * t:2 * t + 2, :],
                                    start=(j in (0, 4) and t == 0),
                                    stop=False,
                                    perf_mode=PM, skip_group_check=True)
                        for j in range(KH):
                            m = KH * ty + j
                            sl = pt[:, j * BD:(j + 1) * BD]
                            nc.tensor.matmul(
                                sl, ident8p[:], db2b8[:, m:m + 2, :],
                                start=False, stop=False,
                                perf_mode=PM, skip_group_check=True)
                            for t in range(KH // 2):
                                nc.tensor.matmul(
                                    sl, w2i8[:, m, 2 * t:2 * t + 2, :],
                                    h1f8[:, 2 * t:2 * t + 2, :],
                                    start=False,
                                    stop=(j == KH - 1 and t == 3),
                                    perf_mode=PM, skip_group_check=True)
                        func = AF.Tanh if ty == 2 else AF.Sigmoid
                        nc.scalar.activation(
                            gates2[:, ty * 1024:(ty + 1) * 1024], pt[:], func)

                    # ---- L2 cell (2 chunk groups) ----
                    for q in range(2):
                        w = slice(q * 512, (q + 1) * 512)
                        gi = gates2[:, q * 512:(q + 1) * 512]
                        gf = gates2[:, 1024 + q * 512:1024 + (q + 1) * 512]
                        gg = gates2[:, 2048 + q * 512:2048 + (q + 1) * 512]
                        go = gates2[:, 3072 + q * 512:3072 + (q + 1) * 512]
                        nc.vector.tensor_mul(tmp[:, w], gi, gg)
                        nc.gpsimd.tensor_mul(c2[:, w], c2[:, w], gf)
                        nc.gpsimd.tensor_add(c2[:, w], c2[:, w], tmp[:, w])
                        nc.scalar.activation(th2[:, w], c2[:, w], AF.Tanh)
                        nc.vector.tensor_mul(h2bf[:, w], go, th2[:, w])
                        nc.gpsimd.tensor_copy(
                            h2f8[:, 4 * q:4 * (q + 1), :].rearrange(
                                'p a b -> p (a b)'), h2bf[:, w])

                    if s % FCB == FCB - 1:
                        emit_fc(s)
                        flush_softmax(s - FCB + 1, s)

    nc.compile()
    return nc


# --------------------------------------------------------------------------
# Host-side packing
# --------------------------------------------------------------------------

def _mmajor(w, kin_pad=None):
    kin, g = w.shape
    kp = kin_pad or kin
    nt = g // 128
    wp = np.zeros((kp, g), dtype=np.float32)
    wp[:kin] = w
    out = (wp.reshape(kp // 128, 128, nt, 128)
             .transpose(2, 1, 0, 3)
             .reshape(nt, 128, kp))
    return np.ascontiguousarray(out.astype(bf16))


def _mmajor8(wp, kc):
    g = wp.shape[1]
    nt = g // 128
    out = wp.reshape(kc, 128, nt, 128).transpose(2, 1, 0, 3)
    return np.ascontiguousarray(out.astype(f8))


def _pad_rows(w, rows):
    out = np.zeros((rows, w.shape[1]), dtype=np.float32)
    out[:w.shape[0]] = w
    return out


def _kpack(b):
    return np.ascontiguousarray(b.reshape(-1, 128).T.astype(np.float32))


def _mmajor_b(w, kc, bias=None, dtype=None):
    """[Kin, G] f32 -> [G//128, 128, kc, 128]; optional bias row at
    chunk kc-2 (fp8 pairs) or the last chunk."""
    kin, g = w.shape
    nt = g // 128
    wp = np.zeros((kc * 128, g), dtype=np.float32)
    wp[:kin] = w
    if bias is not None:
        wp[(kc - 2 if (dtype or bf16) == f8 else kc - 1) * 128] = bias
    out = wp.reshape(kc, 128, nt, 128).transpose(2, 1, 0, 3)
    return np.ascontiguousarray(out.astype(dtype or bf16))


def pack_inputs(i):
    m = {}
    pm = lambda a: np.ascontiguousarray(a.transpose(1, 0, 2, 3))
    m["w1h8"] = pm(_mmajor8(_pad_rows(i["dW1h"], KH * 128), KH))
    m["w2h8"] = pm(_mmajor8(_pad_rows(i["dW2h"], KH * 128), KH))
    m["w2i8"] = pm(_mmajor8(_pad_rows(i["dW2i"], KH * 128), KH))
    db2 = (i["db2i"] + i["db2h"]).astype(f8).astype(np.float32)
    bb = np.zeros((128, NT + 1, BD), dtype=np.float32)
    bb[:, :NT, :] = db2.reshape(NT, 128).T[:, :, None]
    m["db2b8"] = bb.astype(f8)
    cb2 = (i["cb2i"] + i["cb2h"]).astype(f8).astype(np.float32)
    cc = np.zeros((128, NT + 1, BL), dtype=np.float32)
    cc[:, :NT, :] = cb2.reshape(NT, 128).T[:, :, None]
    m["cb2b8"] = cc.astype(f8)
    w1x = i["dW1i"][CO:]
    w1x8 = w1x.astype(f8).astype(np.float32)
    w1xr = w1x - w1x8
    wx = np.zeros((KX * 128, G), dtype=np.float32)
    wx[:V] = w1x8
    wx[VP:VP + V] = w1xr
    m["w1i8"] = pm(_mmajor8(wx, KX))
    m["w1c8"] = pm(_mmajor_b(i["dW1i"][:CO], 6,
                             i["db1i"] + i["db1h"], f8))
    c8 = np.zeros((128, 2, BD), dtype=np.float32)
    c8[0, 0, :] = 1.0
    m["const8"] = c8.astype(f8)
    id8 = np.zeros((128, 2, 128), dtype=np.float32)
    id8[:, 0, :] = np.eye(128, dtype=np.float32)
    m["ident8p"] = id8.astype(f8)
    m["cw1h"] = _mmajor(i["cW1h"])
    m["cw2i8"] = pm(_mmajor8(_pad_rows(i["cW2i"], KH * 128), KH))
    m["cw2h8p"] = pm(_mmajor8(_pad_rows(i["cW2h"], KH * 128), KH))
    m["cw1i"] = np.ascontiguousarray(_mmajor_b(
        i["cW1i"], 4, i["cb1i"] + i["cb1h"]).transpose(1, 0, 2, 3))
    m["ciw"] = np.ascontiguousarray(_mmajor_b(
        i["ci_W"], KZ + 1, i["ci_b"]).transpose(1, 0, 2, 3))
    m["cow"] = _mmajor_b(i["co_W"], KH + 1, i["co_b"])
    m["diw8"] = np.ascontiguousarray(_mmajor_b(
        i["di_W"], 6, i["di_b"], f8).transpose(1, 0, 2, 3))
    m["fcw"] = np.ascontiguousarray(
        i["fc_W"].reshape(KH, 128, V)).astype(bf16)
    m["fcb"] = i["fc_b"].reshape(1, V).astype(bf16)
    m["ones"] = np.ones((1, 128), dtype=bf16)
    m["ident"] = np.eye(128, dtype=bf16)
    return m


def pack_core(i, core):
    s = slice(core * BL, (core + 1) * BL)
    m = {}
    zt = np.zeros(((KZ + 1) * 128, BL), dtype=np.float32)
    zt[:Z] = i["z"][s].T
    zt[Z] = 1.0
    m["zt"] = np.ascontiguousarray(zt.reshape(KZ + 1, 128, BL).astype(bf16))
    ci = np.zeros(((KV + 1) * 128, BL), dtype=np.float32)
    ci[:V] = i["conductor_input"][0, s].T
    ci[VP] = 1.0
    m["cin"] = np.ascontiguousarray(ci.reshape(KV + 1, 128, BL).astype(bf16))
    x = i["x"]
    prev = np.zeros((SUB, VP, U, BL), dtype=np.float32)
    for t in range(SUB):
        for u in range(U):
            sq = u * SUB + t
            if sq == 0:
                prev[t, 0, u, :] = 1.0
            else:
                prev[t, :V, u, :] = x[sq - 1, s].T
    pc = prev.reshape(SUB, KV, 128, U * BL)
    m["prev8"] = np.ascontiguousarray(
        np.concatenate([pc, pc], axis=1).transpose(0, 2, 1, 3).astype(f8))
    return m


_NC_CACHE = {}


def _get_nc(key=(U, SUB)):
    if key not in _NC_CACHE:
        _NC_CACHE[key] = build_nc(*key)
    return _NC_CACHE[key]


def kernel(**inputs):
    inputs = {k: np.asarray(v) for k, v in inputs.items()}
    nc = _get_nc()
    shared = pack_inputs(inputs)
    in_maps = [dict(shared, **pack_core(inputs, c)) for c in range(NCORES)]
    r = run_bass_kernel_spmd(nc, in_maps, core_ids=list(range(NCORES)))
    out = np.concatenate([r.results[c]["out"] for c in range(NCORES)], axis=0)
    return out.astype(np.float32)
